# revision 3
# baseline (speedup 1.0000x reference)
"""Trainium2 Bass kernel for a single-head causal self-attention variant (v2).

Reference semantics (B=4, S=2048, D=1024):
    q = x @ wq.T ; k = x @ wk.T ; v = x @ wv.T
    scores = q @ k.T / sqrt(D)          # [B, S, S]
    a = softmax(scores, axis=-2)        # softmax over the QUERY axis, per key column
    a = triu(a)                         # keep q <= k, applied AFTER softmax
    out = a.T @ v                       # out row k = sum_{q<=k} a[q,k] * v[q]

Algebraic folds:
  * scores = x @ (wq.T @ wk) @ x.T -> wq/wk fold into MT = (wk.T @ wq)/sqrt(D)
    on the host; KM = MT.T-contract with x.T gives the "key side" once.
  * No max-subtraction needed (scores are O(1)); denominators (colsum over ALL
    q) divide the output rows on the host.
  * Output path folds wv AFTER the masked transpose-accumulate:
        U = (Emask.T @ x) @ wv.T
    so the V projection (which the old kernel computed REDUNDANTLY on both
    half-cores, 256 extra matmuls) disappears; the new TT = Emask.T @ x costs
    barely more than the old Emask.T @ V and the final TT @ wv.T touches only
    the core's LOCAL 1024 k rows.

Sharding (8 cores): core = (batch b = core//2, half h = core%2); core owns
interleaved k blocks 2j+h (j=0..7) of its batch, balancing triangular work.

All matmuls are bf16 (rel-l2 error ~1.3e-3, gate is 2e-2): unlike fp32r,
bf16 weights load via a separate LDWEIGHTS instruction that the PE's 64-deep
reorder window pulls ahead of in-flight matmuls, so the 128-col weight load
overlaps the previous matmul's 512-col stream instead of serializing after it.

Per-core PE stream: 128 KM + 256 E + 32 colsum + 128 Twv N=512 matmuls plus
320 N=256 ATx matmuls = ~360k PE cycles (150us at 2.4GHz; the fp32r baseline
was 816 matmuls at 640 cycles = 216us).

SPMD uniformity: h=0 and h=1 cores run the SAME program; the causal-mask
geometry lives in a per-core "bigmask" input ([128, 16*256]: for each q-chunk
qc, a 256-wide mask over the 2 local k blocks of window jp=qc//4 with
0/triu/1 per block below/on/above the diagonal).  The handful of matmuls this
wastes on h=0 keeps both halves in one NEFF.  ATx streams at 256-column
granularity (4 windows of 2 local k blocks) so the causal triangle skips
more dead work than a 512-wide window would.
"""

import numpy as np

B, S, D = 4, 2048, 1024
P = 128
SK = 1024          # k columns per core
KD = D // P        # 8 contraction chunks
NB = S // P        # 16 q chunks / global k blocks
NJ = SK // P       # 8 local k chunks
NCORES = 8

_cache = {}


def _build_module(reps=1, accum=False):
    import concourse.bacc as bacc
    import concourse.tile as tile
    from concourse import mybir

    f32 = mybir.dt.float32
    bf = mybir.dt.bfloat16
    Exp = mybir.ActivationFunctionType.Exp

    nc = bacc.Bacc("TRN2", target_bir_lowering=False, debug=False,
                   num_devices=NCORES)

    xT = nc.dram_tensor("xT", [D, S], bf, kind="ExternalInput").ap()
    xtk = nc.dram_tensor("xtk", [D, SK], bf, kind="ExternalInput").ap()
    xn = nc.dram_tensor("xn", [S, D], bf, kind="ExternalInput").ap()
    mt = nc.dram_tensor("mt", [D, D], bf, kind="ExternalInput").ap()
    wvT = nc.dram_tensor("wvT", [D, D], bf, kind="ExternalInput").ap()
    bigmask = nc.dram_tensor("bigmask", [P, NB * 256], bf,
                             kind="ExternalInput").ap()
    onesd = nc.dram_tensor("onesd", [P, 1], bf, kind="ExternalInput").ap()
    out = nc.dram_tensor("out", [SK, D], f32, kind="ExternalOutput").ap()
    cso = nc.dram_tensor("cso", [1, SK], f32, kind="ExternalOutput").ap()

    mm = nc.tensor.matmul

    with tile.TileContext(nc) as tc:
        from contextlib import ExitStack
        for _rep in range(reps):
          with ExitStack() as ctx:
            persist = ctx.enter_context(tc.tile_pool(name="persist", bufs=1))
            psum = ctx.enter_context(tc.tile_pool(name="psum", bufs=2,
                                                  space="PSUM"))

            ones_t = persist.tile([P, 1], bf, tag="ones")
            nc.sync.dma_start(ones_t, onesd)
            bm_t = persist.tile([P, NB, 256], bf, tag="bm")
            for qc in range(NB):
                nc.sync.dma_start(bm_t[:, qc, :],
                                  bigmask[:, qc * 256:(qc + 1) * 256])
            wv_t = persist.tile([P, KD, D], bf, tag="wv")
            for c in range(KD):
                nc.sync.dma_start(wv_t[:, c, :], wvT[c * P:(c + 1) * P, :])
            xT_t = persist.tile([P, KD, S], bf, tag="xT")
            for c in range(KD):
                nc.sync.dma_start(xT_t[:, c, :], xT[c * P:(c + 1) * P, :])

            km_t = persist.tile([P, KD, SK], bf, tag="km")
            tt_t = persist.tile([P, KD, SK], bf, tag="tt")

            # ---- phase K: KM[dq, k] = sum_dk MT[dk, dq] * xtk[dk, k] ----
            with tc.tile_pool(name="pk", bufs=1) as pk:
                mt_t = pk.tile([P, KD, D], bf, tag="mt")
                xtk_t = pk.tile([P, KD, SK], bf, tag="xtk")
                for c in range(KD):
                    nc.sync.dma_start(mt_t[:, c, :], mt[c * P:(c + 1) * P, :])
                    nc.sync.dma_start(xtk_t[:, c, :],
                                      xtk[c * P:(c + 1) * P, :])
                for kf in range(2):
                    for dq in range(KD):
                        ps = psum.tile([P, 512], f32, tag="mm", name="ps_km",
                                       bufs=6)
                        for c in range(KD):
                            mm(ps, mt_t[:, c, dq * P:(dq + 1) * P],
                               xtk_t[:, c, kf * 512:(kf + 1) * 512],
                               start=(c == 0), stop=(c == KD - 1))
                        nc.vector.tensor_copy(
                            km_t[:, dq, kf * 512:(kf + 1) * 512], ps)

            # xn reuses the pk pool's address space; its DMAs wait for the
            # last KM matmul reads and complete long before ATx needs them.
            xnp = ctx.enter_context(tc.tile_pool(name="xnp", bufs=1))
            xn_t = xnp.tile([P, NB, D], bf, tag="xn")
            for qc in range(NB):
                nc.sync.dma_start(xn_t[:, qc, :], xn[qc * P:(qc + 1) * P, :])

            # ---- phase E: E = exp(x @ KM), colsum, masked boundary copies --
            cs_ps = [psum.tile([1, 512], f32, tag=f"cs{kf}", name=f"ps_cs{kf}",
                               bufs=1) for kf in range(2)]
            eg = []
            em = []

            def emit_cs(q):
                for kf in range(2):
                    mm(cs_ps[kf], ones_t, eg[q][:, kf * 512:(kf + 1) * 512],
                       start=(q == 0), stop=(q == NB - 1),
                       skip_group_check=True)

            for qc in range(NB):
                egt = persist.tile([P, SK], bf, tag="eg", bufs=NB,
                                   name=f"eg{qc}")
                eg.append(egt)
                for kf in range(2):
                    ps = psum.tile([P, 512], f32, tag="mm", name="ps_e",
                                   bufs=6)
                    for c in range(KD):
                        mm(ps, xT_t[:, c, qc * P:(qc + 1) * P],
                           km_t[:, c, kf * 512:(kf + 1) * 512],
                           start=(c == 0), stop=(c == KD - 1))
                    nc.scalar.activation(
                        egt[:, kf * 512:(kf + 1) * 512], ps, Exp)
                jb = qc // 4
                emt = persist.tile([P, 256], bf, tag="em", bufs=16,
                                   name=f"em{qc}")
                nc.vector.tensor_mul(
                    emt, egt[:, jb * 256:(jb + 1) * 256], bm_t[:, qc, :])
                em.append(emt)
                if qc >= 1:
                    # trailing by one q-chunk keeps the in-order PE stream
                    # from head-of-line blocking on the ACT exp
                    emit_cs(qc - 1)

            # ---- phase ATx: TT[d, k] = sum_q x[q, d] * Emask[q, k] ----
            # 256-wide windows jp (2 local k blocks each); chain jp takes
            # q chunks 0..4*jp+3 (the rest are causally dead)
            for jp in range(4):
                nmem = 4 * jp + 4
                for c in range(KD):
                    ps = psum.tile([P, 256], f32, tag="mm", name="ps_at",
                                   bufs=6)
                    for ql in range(nmem):
                        rhs = em[ql] if ql // 4 == jp else \
                            eg[ql][:, jp * 256:(jp + 1) * 256]
                        mm(ps, xn_t[:, ql, c * P:(c + 1) * P], rhs,
                           start=(ql == 0), stop=(ql == nmem - 1))
                    nc.vector.tensor_copy(
                        tt_t[:, c, jp * 256:(jp + 1) * 256], ps)
                    if jp == 0 and c == 1:
                        emit_cs(NB - 1)

            # ---- phase Twv: U[k, dv] = sum_d TT[d, k] * wvT[d, dv] ----
            up = ctx.enter_context(tc.tile_pool(name="up", bufs=2))
            for kc in range(NJ):
                for dv in range(2):
                    ps = psum.tile([P, 512], f32, tag="mm", name="ps_uv",
                                   bufs=6)
                    for c in range(KD):
                        mm(ps, tt_t[:, c, kc * P:(kc + 1) * P],
                           wv_t[:, c, dv * 512:(dv + 1) * 512],
                           start=(c == 0), stop=(c == KD - 1))
                    ut = up.tile([P, 512], f32, tag="u")
                    nc.vector.tensor_copy(ut, ps)
                    dst = out[kc * P:(kc + 1) * P, dv * 512:(dv + 1) * 512]
                    if accum:
                        nc.gpsimd.dma_start(dst, ut,
                                            accum_op=mybir.AluOpType.add)
                    else:
                        nc.sync.dma_start(dst, ut)

            # ---- epilogue: ship column sums; normalization on host ----
            for kf in range(2):
                cs_sb = persist.tile([1, 512], f32, tag=f"cs_sb{kf}",
                                     name=f"cs_sb{kf}")
                nc.vector.tensor_copy(cs_sb, cs_ps[kf])
                dst = cso[:, kf * 512:(kf + 1) * 512]
                if accum:
                    nc.gpsimd.dma_start(dst, cs_sb,
                                        accum_op=mybir.AluOpType.add)
                else:
                    nc.sync.dma_start(dst, cs_sb)

    nc.compile()
    return nc


def _get_nc(reps=1, accum=False):
    key = ("nc", reps, accum)
    if key not in _cache:
        _cache[key] = _build_module(reps, accum)
    return _cache[key]


def _bigmask(h):
    tri = np.triu(np.ones((P, P), np.float32))
    m = np.zeros((P, NB, 2 * P), np.float32)
    for qc in range(NB):
        jp = qc // 4
        for i in range(2):
            g = 2 * (2 * jp + i) + h
            if g > qc:
                m[:, qc, i * P:(i + 1) * P] = 1.0
            elif g == qc:
                m[:, qc, i * P:(i + 1) * P] = tri
    return m.reshape(P, NB * 256)


def make_in_maps(x, wq, wk, wv):
    import ml_dtypes
    bf16 = ml_dtypes.bfloat16
    x = np.asarray(x, np.float32)
    mt = ((np.asarray(wk, np.float64).T @ np.asarray(wq, np.float64))
          / np.sqrt(float(D))).astype(np.float32)
    wvT = np.ascontiguousarray(np.asarray(wv, np.float32).T)
    in_maps = []
    for core in range(NCORES):
        b, h = core // 2, core % 2
        xb = x[b]
        xTb = np.ascontiguousarray(xb.T)                 # [D, S]
        cols = np.concatenate(
            [np.arange((2 * j + h) * P, (2 * j + h + 1) * P)
             for j in range(NJ)])
        xtkb = np.ascontiguousarray(xTb[:, cols])        # [D, SK]
        m = {
            "xT": xTb, "xtk": xtkb, "xn": xb, "mt": mt, "wvT": wvT,
            "bigmask": _bigmask(h), "onesd": np.ones((P, 1), np.float32),
        }
        in_maps.append({k: v.astype(bf16) for k, v in m.items()})
    return in_maps


def gather(results):
    full = np.empty((B, S, D), np.float32)
    for core in range(NCORES):
        b, h = core // 2, core % 2
        o = results[core]["out"] / results[core]["cso"][0][:, None]
        for j in range(NJ):
            full[b, (2 * j + h) * P:(2 * j + h + 1) * P, :] = \
                o[j * P:(j + 1) * P, :]
    return full


def kernel(x, wq, wk, wv):
    from concourse.bass_utils import run_bass_kernel_spmd
    nc = _get_nc()
    in_maps = make_in_maps(x, wq, wk, wv)
    res = run_bass_kernel_spmd(nc, in_maps, core_ids=list(range(NCORES)))
    return gather(res.results)
